# revision 1
# baseline (speedup 1.0000x reference)
"""CrossTransformer Trainium2 kernel — 8 NeuronCores.

Sharding: core c = (batch b = c//2, head-pair hg = c%2).  Attention is
head-parallel (2 heads/core, fp32r matmuls, exp on ACT with fused
row-sum accum); out-proj + FFN are token-parallel (half of the 2048
tokens per core) after an intra-pair AllToAll of the attention output.

Both softmax directions run the same "column-softmax" pipeline with
swapped inputs (m0 = m1-path(x1, x0)); each path's softmax denominator
is the opposite path's exp row-sum (E_ba = E_ab^T).
"""
import numpy as np

B, NT, E, H, D = 4, 2048, 256, 4, 64
HPC = 2            # heads per core
TH = NT // 2       # token half
HID = 2 * E        # FFN hidden (512)
KCH = E // 128     # 128-chunks of E (2)
N_CORES = 8
LN_EPS = 1e-5

_cache = {}


def _build():
    import concourse.bass as bass
    import concourse.tile as tile
    from concourse import bacc
    import concourse.mybir as mybir

    dt = mybir.dt
    AF = mybir.ActivationFunctionType
    OP = mybir.AluOpType
    f32, f32r = dt.float32, dt.float32r

    nc = bacc.Bacc("TRN2", target_bir_lowering=False, debug=False,
                   num_devices=N_CORES)

    def din(name, shape):
        return nc.dram_tensor(name, shape, f32, kind="ExternalInput").ap()

    x0t = din("x0t", [E, NT])          # x0[b].T
    x1t = din("x1t", [E, NT])
    xf_r = [din(f"x{d}t_ffn", [E, TH]) for d in (0, 1)]   # my token half
    wqk = din("wqk", [E, 128])         # pre-scaled, this core's heads
    bqk = din("bqk", [128, 1])
    wv = din("wv", [E, 256])
    bv = din("bv", [128, 1])
    wo = din("wo", [E, E])
    bo = din("bo", [E, 1])
    w1 = din("w1", [HID, HID])
    w1bar = din("w1bar", [HID, 1])
    b1 = din("b1", [HID, 1])
    b1bar = din("b1bar", [1, 1])
    lng = din("lng", [HID, 1])
    lnb = din("lnb", [HID, 1])
    w2 = din("w2", [HID, E])
    b2 = din("b2", [E, 1])
    ident = din("ident", [128, 128])   # identity matrix
    ones = din("ones", [128, 1])

    outs = [nc.dram_tensor(f"out{d}t", [E, TH], f32, kind="ExternalOutput").ap()
            for d in (0, 1)]

    with tile.TileContext(nc) as tc:
        _body(nc, tc, bass, mybir, tile,
              dict(x0t=x0t, x1t=x1t, xf_r=xf_r, wqk=wqk, bqk=bqk, wv=wv,
                   bv=bv, wo=wo, bo=bo, w1=w1, w1bar=w1bar, b1=b1,
                   b1bar=b1bar, lng=lng, lnb=lnb, w2=w2, b2=b2,
                   ident=ident, ones=ones, outs=outs))
    nc.compile()
    return nc


def _body(nc, tc, bass, mybir, tile, t):
    from contextlib import ExitStack
    dt = mybir.dt
    AF = mybir.ActivationFunctionType
    OP = mybir.AluOpType
    f32, f32r = dt.float32, dt.float32r

    es = ExitStack()
    with es:
        wpool = es.enter_context(tc.tile_pool(name="weights", bufs=1))
        dram = es.enter_context(tc.tile_pool(name="dram", bufs=1, space="DRAM"))

        # ---- weight loads (SWDGE casts fp32 -> fp32r where PE consumes) ----
        def load_r(ap_src, p, fshape, tag):
            til = wpool.tile([p, fshape], f32r, tag=tag, name=tag)
            nc.gpsimd.dma_start(til[:], ap_src)
            return til

        def load_f(ap_src, p, fshape, tag):
            til = wpool.tile([p, fshape], f32, tag=tag, name=tag)
            nc.sync.dma_start(til[:], ap_src)
            return til

        wqk_t = [load_r(t["wqk"][k * 128:(k + 1) * 128, :], 128, 128, f"wqk{k}") for k in range(KCH)]
        wv_t = [load_r(t["wv"][k * 128:(k + 1) * 128, :], 128, 256, f"wv{k}") for k in range(KCH)]
        wo_t = [[load_r(t["wo"][k * 128:(k + 1) * 128, m * 128:(m + 1) * 128], 128, 128, f"wo{k}{m}")
                 for m in range(2)] for k in range(KCH)]
        w1_t = [[load_r(t["w1"][k * 128:(k + 1) * 128, m * 128:(m + 1) * 128], 128, 128, f"w1{k}{m}")
                 for m in range(4)] for k in range(4)]
        w2_t = [[load_r(t["w2"][k * 128:(k + 1) * 128, m * 128:(m + 1) * 128], 128, 128, f"w2{k}{m}")
                 for m in range(2)] for k in range(4)]
        w1bar_t = [load_r(t["w1bar"][k * 128:(k + 1) * 128, :], 128, 1, f"w1b{k}") for k in range(4)]
        ones_t = load_r(t["ones"], 128, 1, "ones")
        ident_t = load_r(t["ident"], 128, 128, "ident")
        bqk_t = load_f(t["bqk"], 128, 1, "bqk")
        bv_t = load_f(t["bv"], 128, 1, "bv")
        bo_t = [load_f(t["bo"][m * 128:(m + 1) * 128, :], 128, 1, f"bo{m}") for m in range(2)]
        b1_t = [load_f(t["b1"][m * 128:(m + 1) * 128, :], 128, 1, f"b1_{m}") for m in range(4)]
        b1bar_t = load_f(t["b1bar"], 1, 1, "b1bar")
        lng_t = [load_f(t["lng"][m * 128:(m + 1) * 128, :], 128, 1, f"lng{m}") for m in range(4)]
        lnb_t = [load_f(t["lnb"][m * 128:(m + 1) * 128, :], 128, 1, f"lnb{m}") for m in range(4)]
        b2_t = [load_f(t["b2"][m * 128:(m + 1) * 128, :], 128, 1, f"b2_{m}") for m in range(2)]

        xffn_r = [[None, None], [None, None]]
        xffn_f = [[None, None], [None, None]]
        for d in range(2):
            for k in range(KCH):
                sl = t["xf_r"][d][k * 128:(k + 1) * 128, :]
                xffn_r[d][k] = load_r(sl, 128, TH, f"xfr{d}{k}")
                xffn_f[d][k] = load_f(sl, 128, TH, f"xff{d}{k}")

        # ================= projections =================
        qkT = [None, None]   # [128(2h*64d), NT] fp32r
        v_t = [[None] * 16, [None] * 16]   # 16 x [128 tok, 128(2h*64d)]
        rows = es.enter_context(tc.tile_pool(name="rows", bufs=1))
        attn_es = ExitStack()
        qp = attn_es.enter_context(tc.tile_pool(name="qkv", bufs=1))
        with tc.tile_pool(name="xfull", bufs=1) as xp, \
             tc.tile_pool(name="qkps", bufs=1, space="PSUM") as qkps, \
             tc.tile_pool(name="vps", bufs=3, space="PSUM") as vps:
            xt = [[None, None], [None, None]]
            for s, src in enumerate((t["x0t"], t["x1t"])):
                for k in range(KCH):
                    xt[s][k] = xp.tile([128, NT], f32r, tag=f"x{s}{k}", name=f"x{s}{k}")
                    nc.gpsimd.dma_start(xt[s][k][:], src[k * 128:(k + 1) * 128, :])
            for s in range(2):
                ps = qkps.tile([128, NT], f32)
                for jn in range(NT // 512):
                    for k in range(KCH):
                        nc.tensor.matmul(ps[:, jn * 512:(jn + 1) * 512],
                                         wqk_t[k][:], xt[s][k][:, jn * 512:(jn + 1) * 512],
                                         start=(k == 0), stop=(k == KCH - 1))
                qkT[s] = qp.tile([128, NT], f32r, tag=f"qkT{s}", name=f"qkT{s}")
                nc.scalar.activation(qkT[s][:], ps[:], AF.Identity, bias=bqk_t[:], scale=1.0)
            for s in range(2):
                for it in range(16):
                    pv = vps.tile([128, 256], f32, tag="vps")
                    for var in range(2):
                        for k in range(KCH):
                            nc.tensor.matmul(pv[:, var * 128:(var + 1) * 128],
                                             xt[s][k][:, it * 128:(it + 1) * 128],
                                             wv_t[k][:, var * 128:(var + 1) * 128],
                                             start=(k == 0), stop=(k == KCH - 1))
                    v_t[s][it] = qp.tile([128, 256], f32r, tag=f"v{s}_{it}", name=f"v{s}_{it}")
                    nc.vector.tensor_copy(v_t[s][it][:], pv[:])

        # ================= attention (two symmetric paths) =================
        # path p: (A,B) = (p, 1-p); output = m for dir (1-p) tokens of x_{1-p}
        mn_pool = attn_es.enter_context(tc.tile_pool(name="mnorm", bufs=1))
        rsj = [rows.tile([128, 64], f32, tag=f"rsj{jj}", name=f"rsj{jj}") for jj in range(2)]
        # row-vector tiles: engine ops need base partition 0 (and equal
        # bases across SBUF operands), so each row vector gets its own tile
        m_raw = [None, None]
        mrp = attn_es.enter_context(tc.tile_pool(name="mraw", bufs=1))
        with tc.tile_pool(name="estrip", bufs=4) as ep, \
             tc.tile_pool(name="simps", bufs=3, space="PSUM") as simps, \
             tc.tile_pool(name="avps", bufs=2, space="PSUM") as avps:
            for p in range(2):
                A, Bi = p, 1 - p
                m_raw[p] = mrp.tile([128, NT], f32, tag=f"mraw{p}", name=f"mraw{p}")
                for jj in range(2):
                    av = [avps.tile([128, 512], f32, tag="av", name=f"av{p}_{jj}_{_i}") for _i in range(2)]
                    for it in range(16):
                        est = [None, None]
                        for h in range(2):
                            sp = simps.tile([128, 1024], f32, tag="sim")
                            for jc in range(2):
                                nc.tensor.matmul(
                                    sp[:, jc * 512:(jc + 1) * 512],
                                    qkT[A][64 * h:64 * (h + 1), it * 128:(it + 1) * 128],
                                    qkT[Bi][64 * h:64 * (h + 1),
                                            jj * 1024 + jc * 512:jj * 1024 + (jc + 1) * 512],
                                    start=True, stop=True,
                                    tile_position=(64 * h, 0))
                            est[h] = ep.tile([128, 1024], f32r, tag="est", name=f"est{h}")
                            col = (p * 2 + h) * 16 + it
                            nc.scalar.activation(est[h][:], sp[:], AF.Exp,
                                                 accum_out=rsj[jj][:, col:col + 1])
                        for jc in range(2):
                            for h in range(2):
                                # lhsT = zero-padded v variant h: rows 64h:64h+64
                                # of the product get head h's AV, rest zeros
                                nc.tensor.matmul(
                                    av[jc][:],
                                    v_t[A][it][:, h * 128:(h + 1) * 128],
                                    est[h][:, jc * 512:(jc + 1) * 512],
                                    start=(it == 0 and h == 0),
                                    stop=(it == 15 and h == 1))
                    for jc in range(2):
                        nc.vector.tensor_copy(
                            m_raw[p][:, jj * 1024 + jc * 512:jj * 1024 + (jc + 1) * 512],
                            av[jc][:])

        # ---- denominators: den[path p] = rowsums of path (1-p) ----
        rsall = rows.tile([128, 64], f32, tag="rsall")
        nc.vector.tensor_add(rsall[:], rsj[0][:], rsj[1][:])
        rsall_r = rows.tile([128, 64], f32r, tag="rsallr")
        nc.vector.tensor_copy(rsall_r[:], rsall[:])
        with tc.tile_pool(name="trps", bufs=1, space="PSUM") as trps:
            tp = trps.tile([64, 128], f32)
            nc.tensor.matmul(tp[:], rsall_r[:], ident_t[:], start=True, stop=True)
            rsT = rows.tile([64, 128], f32, tag="rsT")
            nc.vector.tensor_copy(rsT[:], tp[:])
        den_dram = dram.tile([4, 2048], f32)
        for r in range(4):
            nc.sync.dma_start(
                den_dram[r].rearrange("(it p) -> it p", it=16),
                rsT[r * 16:(r + 1) * 16, :])
        den_rows = mn_pool.tile([4, 2048], f32, tag="denrows", name="denrows")
        nc.sync.dma_start(den_rows[:], den_dram[:])
        lnden = mn_pool.tile([4, 2048], f32, tag="lnden", name="lnden")
        nc.scalar.activation(lnden[:], den_rows[:], AF.Ln)
        recipden = mn_pool.tile([4, 2048], f32, tag="recipden", name="recipden")
        nc.scalar.activation(recipden[:], lnden[:], AF.Exp, scale=-1.0)
        recip_dram = dram.tile([4, 2048], f32)
        nc.sync.dma_start(recip_dram[:], recipden[:])

        # ---- normalize + bv;  den for path p = rows (1-p)*2+h ----
        # DVE cannot take 0-step partition APs, so materialize the row
        # broadcast with a DMA from DRAM (partitions 64h:64h+64 <- head h row).
        m_norm = [None, None]
        for p in range(2):
            recipb = mn_pool.tile([128, NT], f32, tag=f"recipb{p}", name=f"recipb{p}")
            for h in range(2):
                r = (1 - p) * 2 + h
                nc.sync.dma_start(recipb[64 * h:64 * (h + 1), :],
                                  recip_dram[r:r + 1, :].to_broadcast((64, NT)))
            m_norm[p] = mn_pool.tile([128, NT], f32, tag=f"mnorm{p}", name=f"mnorm{p}")
            nc.vector.tensor_mul(m_norm[p][:], m_raw[p][:], recipb[:])
            nc.vector.tensor_scalar(m_norm[p][:], m_norm[p][:], bv_t[:], None, OP.add)

        # ======== exchange: 8-way AllToAll, reshard (b,hg) -> token-eighth ====
        # block r (of 8) = token columns [r*256:(r+1)*256]; after the
        # exchange, core c holds m for ALL batches at ITS 256-token slice.
        # bounce layout: [8 blocks, 2 paths, 128, 256]
        bounce_in = dram.tile([4, 2, 2, 128, 256], f32)   # (b_blk, kc_blk, path, p, t)
        bounce_out = dram.tile([4, 2, 2, 128, 256], f32)
        for p in range(2):
            for bb in range(4):
                for kb in range(2):
                    nc.sync.dma_start(
                        bounce_in[bb, kb, p],
                        m_norm[p][:, (2 * bb + kb) * 256:(2 * bb + kb + 1) * 256])
        nc.gpsimd.collective_compute(
            "AllToAll", mybir.AluOpType.bypass,
            replica_groups=[list(range(8))],
            ins=[bounce_in.opt()], outs=[bounce_out.opt()])
        attn_es.close()   # frees qkT/v/m_raw/m_norm SBUF for the FFN phase
        # out block s = from core s=(b=s//2, hg=s%2): m[batch b, heads hg, my toks]
        # m_dir[d][kc][:, b*256:(b+1)*256] = bounce_out[2b+kc, 1-d]
        mdir = [[None, None], [None, None]]   # [dir][kc] -> [128, TH=4x256] f32r
        mpool = es.enter_context(tc.tile_pool(name="mdir", bufs=1))
        for d in range(2):
            p = 1 - d
            for kc in range(2):
                mdir[d][kc] = mpool.tile([128, TH], f32r, tag=f"mdir{d}{kc}", name=f"mdir{d}{kc}")
                for bb in range(4):
                    nc.gpsimd.dma_start(mdir[d][kc][:, bb * 256:(bb + 1) * 256],
                                        bounce_out[bb, kc, p])

        # ================= out-projection =================
        mproj = [[None, None], [None, None]]
        with tc.tile_pool(name="mpps", bufs=2, space="PSUM") as mpps:
            for d in range(2):
                for mo in range(2):
                    ps = mpps.tile([128, TH], f32, tag="mp")
                    for nn in range(2):
                        for kc in range(2):
                            nc.tensor.matmul(ps[:, nn * 512:(nn + 1) * 512],
                                             wo_t[kc][mo][:],
                                             mdir[d][kc][:, nn * 512:(nn + 1) * 512],
                                             start=(kc == 0), stop=(kc == 1))
                    mproj[d][mo] = mpool.tile([128, TH], f32r, tag=f"mproj{d}{mo}", name=f"mproj{d}{mo}")
                    nc.scalar.activation(mproj[d][mo][:], ps[:], AF.Identity,
                                         bias=bo_t[mo][:], scale=1.0)

        # ================= FFN =================
        # ccT chunks (f32r): [xffn_r[d][0], xffn_r[d][1], mproj[d][0], mproj[d][1]]
        hsb_pool = es.enter_context(tc.tile_pool(name="hsb", bufs=1))
        hsb = {}
        statp = es.enter_context(tc.tile_pool(name="statrows", bufs=1))
        mu_all = statp.tile([1, 2048], f32, tag="muall", name="muall")
        ss_all = statp.tile([1, 2048], f32, tag="srowA", name="ssall")
        with tc.tile_pool(name="sq", bufs=3) as sqp, \
             tc.tile_pool(name="hps", bufs=4, space="PSUM") as hps, \
             tc.tile_pool(name="rowps", bufs=2, space="PSUM") as rowps:
            for d in range(2):
                cc = [xffn_r[d][0], xffn_r[d][1], mproj[d][0], mproj[d][1]]
                for tcn in range(2):
                    sl = slice(tcn * 512, (tcn + 1) * 512)
                    col = (d * 2 + tcn) * 512
                    pmu = rowps.tile([1, 512], f32, tag="pmu")
                    for kc in range(4):
                        nc.tensor.matmul(pmu[:], w1bar_t[kc][:], cc[kc][:, sl],
                                         start=(kc == 0), stop=(kc == 3))
                    nc.vector.tensor_scalar(mu_all[0:1, col:col + 512], pmu[:],
                                            b1bar_t[:], None, OP.add)
                    pss = rowps.tile([1, 512], f32, tag="pss")
                    for mh in range(4):
                        ph = hps.tile([128, 512], f32, tag="ph")
                        for kc in range(4):
                            nc.tensor.matmul(ph[:], w1_t[kc][mh][:], cc[kc][:, sl],
                                             start=(kc == 0), stop=(kc == 3))
                        hkey = (d, tcn, mh)
                        hsb[hkey] = hsb_pool.tile([128, 512], f32, tag=f"h{d}{tcn}{mh}", name=f"h{d}{tcn}{mh}")
                        nc.vector.tensor_scalar(hsb[hkey][:], ph[:], b1_t[mh][:],
                                                None, OP.add)
                        sq = sqp.tile([128, 512], f32r, tag="sq")
                        nc.vector.tensor_mul(sq[:], hsb[hkey][:], hsb[hkey][:])
                        nc.tensor.matmul(pss[:], ones_t[:], sq[:],
                                         start=(mh == 0), stop=(mh == 3))
                    nc.vector.tensor_copy(ss_all[0:1, col:col + 512], pss[:])

        # batched LN stats: rstd = exp(-0.5 ln(ss/512 - mu^2 + eps))
        musq = statp.tile([1, 2048], f32, tag="srowB", name="musq")
        nc.vector.tensor_mul(musq[:], mu_all[:], mu_all[:])
        ve = statp.tile([1, 2048], f32, tag="srowC", name="ve")
        nc.vector.scalar_tensor_tensor(ve[:], ss_all[:], 1.0 / HID, musq[:],
                                       OP.mult, OP.subtract)
        vee = statp.tile([1, 2048], f32, tag="srowA", name="vee")
        nc.vector.tensor_scalar(vee[:], ve[:], LN_EPS, None, OP.add)
        lnve = statp.tile([1, 2048], f32, tag="srowB", name="lnve")
        nc.scalar.activation(lnve[:], vee[:], AF.Ln)
        rstd = statp.tile([1, 2048], f32, tag="srowA", name="rstd")
        nc.scalar.activation(rstd[:], lnve[:], AF.Exp, scale=-0.5)
        murstd = statp.tile([1, 2048], f32, tag="srowB", name="murstd")
        nc.vector.tensor_mul(murstd[:], mu_all[:], rstd[:])
        # materialize partition-broadcasts of rstd/murstd via DRAM
        stat_dram = dram.tile([2, 2048], f32)
        nc.sync.dma_start(stat_dram[0:1, :], rstd[:])
        nc.sync.dma_start(stat_dram[1:2, :], murstd[:])
        statb = es.enter_context(tc.tile_pool(name="statb", bufs=1))
        rstdb = statb.tile([128, 2048], f32, tag="rstdb", name="rstdb")
        murstdb = statb.tile([128, 2048], f32, tag="murstdb", name="murstdb")
        nc.sync.dma_start(rstdb[:], stat_dram[0:1, :].to_broadcast((128, 2048)))
        nc.sync.dma_start(murstdb[:], stat_dram[1:2, :].to_broadcast((128, 2048)))

        # affine + gelu + W2 + residual
        with tc.tile_pool(name="uacts", bufs=3) as up, \
             tc.tile_pool(name="gacts", bufs=5) as gp, \
             tc.tile_pool(name="osb", bufs=4) as op_, \
             tc.tile_pool(name="ops", bufs=2, space="PSUM") as ops:
            for d in range(2):
                for tcn in range(2):
                    sl = slice(tcn * 512, (tcn + 1) * 512)
                    col = (d * 2 + tcn) * 512
                    rsl = rstdb[:, col:col + 512]
                    msl = murstdb[:, col:col + 512]
                    gh = [None] * 4
                    for mh in range(4):
                        u = up.tile([128, 512], f32, tag="u")
                        nc.vector.tensor_mul(u[:], hsb[(d, tcn, mh)][:], rsl[:])
                        t2 = up.tile([128, 512], f32, tag="t2")
                        nc.vector.tensor_sub(t2[:], u[:], msl[:])
                        gh[mh] = gp.tile([128, 512], f32r, tag="gh", name=f"gh{mh}")
                        nc.scalar.activation(gh[mh][:], t2[:], AF.Gelu,
                                             bias=lnb_t[mh][:], scale=lng_t[mh][:])
                    for mo in range(2):
                        po = ops.tile([128, 512], f32, tag="po")
                        for kh in range(4):
                            nc.tensor.matmul(po[:], w2_t[kh][mo][:], gh[kh][:],
                                             start=(kh == 0), stop=(kh == 3))
                        ot = op_.tile([128, 512], f32, tag="ot")
                        nc.vector.scalar_tensor_tensor(
                            ot[:], po[:], b2_t[mo][:], xffn_f[d][mo][:, sl],
                            OP.add, OP.add)
                        nc.sync.dma_start(t["outs"][d][mo * 128:(mo + 1) * 128, sl], ot[:])


def _host_prep(inputs):
    """Build per-core in_maps from full inputs."""
    x0 = np.asarray(inputs["x0"], np.float32)
    x1 = np.asarray(inputs["x1"], np.float32)
    Wqk = np.asarray(inputs["Wqk"], np.float32) * (D ** -0.25)
    bqk = np.asarray(inputs["bqk"], np.float32) * (D ** -0.25)
    Wv = np.asarray(inputs["Wv"], np.float32)
    bv = np.asarray(inputs["bv"], np.float32)
    Wo = np.asarray(inputs["Wo"], np.float32)
    bo = np.asarray(inputs["bo"], np.float32)
    W1 = np.asarray(inputs["W1"], np.float32)
    b1 = np.asarray(inputs["b1"], np.float32)
    lng = np.asarray(inputs["ln_g"], np.float32)
    lnb = np.asarray(inputs["ln_b"], np.float32)
    W2 = np.asarray(inputs["W2"], np.float32)
    b2 = np.asarray(inputs["b2"], np.float32)

    shared = {
        "wo": np.ascontiguousarray(Wo),
        "bo": bo.reshape(E, 1),
        "w1": np.ascontiguousarray(W1),
        "w1bar": W1.mean(axis=1).reshape(HID, 1),
        "b1": b1.reshape(HID, 1),
        "b1bar": np.array([[b1.mean()]], np.float32),
        "lng": lng.reshape(HID, 1),
        "lnb": lnb.reshape(HID, 1),
        "w2": np.ascontiguousarray(W2),
        "b2": b2.reshape(E, 1),
        "ident": np.eye(128, dtype=np.float32),
        "ones": np.ones((128, 1), np.float32),
    }
    in_maps = []
    for c in range(N_CORES):
        b, hg = c // 2, c % 2
        hs = slice(hg * 128, hg * 128 + 128)
        ts = slice(hg * TH, hg * TH + TH)
        m = dict(shared)
        m["x0t"] = np.ascontiguousarray(x0[b].T)
        m["x1t"] = np.ascontiguousarray(x1[b].T)
        # FFN slice: my 256-token slice of EVERY batch, columns (b, t) b-major
        cs = slice(c * 256, (c + 1) * 256)
        m["x0t_ffn"] = np.ascontiguousarray(
            x0[:, cs, :].reshape(B * 256, E).T)
        m["x1t_ffn"] = np.ascontiguousarray(
            x1[:, cs, :].reshape(B * 256, E).T)
        m["wqk"] = np.ascontiguousarray(Wqk[:, hs])
        m["bqk"] = bqk[hs].reshape(128, 1)
        wvp = np.zeros((E, 256), np.float32)
        wvp[:, 0:64] = Wv[:, hg * 128:hg * 128 + 64]        # head0 -> cols 0:64
        wvp[:, 192:256] = Wv[:, hg * 128 + 64:hg * 128 + 128]  # head1 -> cols 192:256
        m["wv"] = wvp
        m["bv"] = bv[hs].reshape(128, 1)
        in_maps.append(m)
    return in_maps


def _get_nc():
    if "nc" not in _cache:
        _cache["nc"] = _build()
    return _cache["nc"]


def kernel(**inputs):
    from concourse import bass_utils
    nc = _get_nc()
    in_maps = _host_prep(inputs)
    res = bass_utils.run_bass_kernel_spmd(nc, in_maps, core_ids=list(range(N_CORES)))
    out0 = np.empty((B, NT, E), np.float32)
    out1 = np.empty((B, NT, E), np.float32)
    for c in range(N_CORES):
        cs = slice(c * 256, (c + 1) * 256)
        o0 = res.results[c]["out0t"]  # [E, 4*256], cols (b, t)
        o1 = res.results[c]["out1t"]
        for b in range(B):
            out0[b, cs, :] = o0[:, b * 256:(b + 1) * 256].T
            out1[b, cs, :] = o1[:, b * 256:(b + 1) * 256].T
    return out0, out1

